# revision 1
# baseline (speedup 1.0000x reference)
"""Trainium2 Bass kernel for CheemsNonWoAttention (GQA attention, no output proj).

Sharding: 16 q-heads across 8 cores (2 q-heads + their shared kv-head per
core), SPMD with no collectives.  Each core computes its slice of the output
hidden dim; the host concatenates.

Math notes:
  - The reference's logn scale is max(log(65..80)/log(256), 1) == 1.0 -> no-op.
  - 1/sqrt(HD) score scale is folded into Wq on the host.
  - Softmax runs without max-subtraction (scores ~ N(0,1) + additive mask;
    exp underflows to 0 for very negative masks, which is exactly right).
    Scores are computed transposed, sT[k, q], so exp(sT) feeds attn@V
    directly as the moving operand (V chunks stationary), denominators come
    from a ones-vector matmul, and only the final [d, q] -> [q, d] flip
    needs PE transposes.
  - Matmuls run in float32r (TF32-like, ~1.5e-4 rms rel err per dot;
    ~3e-4 rms end-to-end).  fp32r moving dims must be even and >= 256 for
    full rate; weight loads are the per-matmul bottleneck, so walrus's
    ldw-opt pass is enabled via _patch_ldw_opt (~13% end-to-end).
  - The host inspects the mask and dispatches one of three compiled
    variants: "causal" (skips fully-masked k-chunks and the mask add on
    fully-unmasked ones), "zeros" (no mask work at all), "general"
    (arbitrary additive mask).
"""

import sys

if "/opt/trn_rl_repo" not in sys.path:
    sys.path.insert(0, "/opt/trn_rl_repo")

import math
import numpy as np

B, S, HID = 2, 2048, 2048
NH, NKV, HD = 16, 4, 128
NCORES = 8
HPC = NH // NCORES          # q heads per core
FPC = HPC * HD              # output features per core
KVW = HD                    # kv head width per core
P = 128
NCH = HID // P              # hid chunks (contraction tiles)
TT = 512                    # token tile, phase 1
QT = 512                    # q tile, phase 2
NKC = S // P                # k chunks

_CACHE = {}


def _patch_ldw_opt():
    # walrus's LDWEIGHTS dedup/overlap pass is off by default in the driver
    # args; it is worth ~13% end-to-end here (weight loads dominate fp32r
    # matmul issue otherwise).  Results verified identical with it on.
    import concourse.bass_utils as bu

    if getattr(bu, "_ldw_opt_patched", False):
        return
    orig = bu.run_command

    def patched(argv, **kw):
        argv = ["--enable-ldw-opt=true" if a == "--enable-ldw-opt=false" else a
                for a in argv]
        return orig(argv, **kw)

    bu.run_command = patched
    bu._ldw_opt_patched = True


def _build_nc(variant):
    _patch_ldw_opt()
    import concourse.bacc as bacc
    from concourse import mybir
    from concourse.tile import TileContext

    f32 = mybir.dt.float32
    f32r = mybir.dt.float32r
    bf16 = mybir.dt.bfloat16
    Exp = mybir.ActivationFunctionType.Exp

    nc = bacc.Bacc("TRN2", target_bir_lowering=False, debug=False, num_devices=NCORES)
    xT = nc.dram_tensor("xT", [B, HID, S], f32r, kind="ExternalInput").ap()
    wq = nc.dram_tensor("wq", [HID, FPC], f32r, kind="ExternalInput").ap()
    wk = nc.dram_tensor("wk", [HID, KVW], f32r, kind="ExternalInput").ap()
    wv = nc.dram_tensor("wv", [HID, KVW], f32r, kind="ExternalInput").ap()
    maskT = nc.dram_tensor("maskT", [B, S, S], bf16, kind="ExternalInput").ap()
    ident_d = nc.dram_tensor("ident", [P, P], f32r, kind="ExternalInput").ap()
    ident32_d = nc.dram_tensor("ident32", [P, P], f32, kind="ExternalInput").ap()
    ones_d = nc.dram_tensor("ones", [P, 1], f32r, kind="ExternalInput").ap()
    out = nc.dram_tensor("out", [B, S, FPC], f32, kind="ExternalOutput").ap()

    def active_kchunks(q0):
        if variant == "causal":
            return list(range(q0 // P + QT // P))
        return list(range(NKC))

    def masked_kchunks(q0):
        if variant == "causal":
            return set(range(q0 // P, q0 // P + QT // P))
        if variant == "zeros":
            return set()
        return set(range(NKC))

    with TileContext(nc) as tc:
        with tc.tile_pool(name="persist", bufs=1) as persist:
            wq_sb = persist.tile([P, NCH, FPC], f32r, tag="wq")
            wk_sb = persist.tile([P, NCH, KVW], f32r, tag="wk")
            wv_sb = persist.tile([P, NCH, KVW], f32r, tag="wv")
            ident = persist.tile([P, P], f32r, tag="ident")
            ident32 = persist.tile([P, P], f32, tag="ident32")
            ones_sb = persist.tile([P, 1], f32r, tag="ones")
            qT_sb = [persist.tile([P, HPC, S], f32r, tag=f"qT{b}", name=f"qT{b}") for b in range(B)]
            kT_sb = [persist.tile([P, S], f32r, tag=f"kT{b}", name=f"kT{b}") for b in range(B)]
            v_sb = [persist.tile([P, S], f32r, tag=f"v{b}", name=f"v{b}") for b in range(B)]

            nc.sync.dma_start(out=wq_sb[:], in_=wq.rearrange("(c p) f -> p c f", p=P))
            nc.gpsimd.dma_start(out=wk_sb[:], in_=wk.rearrange("(c p) f -> p c f", p=P))
            nc.gpsimd.dma_start(out=wv_sb[:], in_=wv.rearrange("(c p) f -> p c f", p=P))
            nc.gpsimd.dma_start(out=ident[:], in_=ident_d[:])
            nc.gpsimd.dma_start(out=ident32[:], in_=ident32_d[:])
            nc.gpsimd.dma_start(out=ones_sb[:], in_=ones_d[:])

            # ---------------- phase 1: Q/K/V projections ----------------
            with tc.tile_pool(name="xt", bufs=2) as xpool, \
                 tc.tile_pool(name="vst", bufs=2) as vstage, \
                 tc.tile_pool(name="ppsum", bufs=4, space="PSUM") as ppsum, \
                 tc.tile_pool(name="tpsum", bufs=2, space="PSUM") as tpsum:
                XSUB = 4                      # hid chunks per xt sub-tile
                NSUB = NCH // XSUB
                for b in range(B):
                    for t0 in range(0, S, TT):
                        xts = []
                        for s in range(NSUB):
                            xs = xpool.tile([P, XSUB, TT], f32r, tag=f"xt{s}",
                                            name=f"xt{s}_{b}_{t0}")
                            nc.sync.dma_start(
                                out=xs[:],
                                in_=xT[b, s * XSUB * P:(s + 1) * XSUB * P, t0:t0 + TT]
                                .rearrange("(c p) t -> p c t", p=P),
                            )
                            xts.append(xs)
                        xt = None
                        for h in range(HPC):
                            ps = ppsum.tile([P, TT], f32, tag="pp")
                            for c in range(NCH):
                                nc.tensor.matmul(
                                    ps[:],
                                    lhsT=wq_sb[:, c, h * HD:(h + 1) * HD],
                                    rhs=xts[c // XSUB][:, c % XSUB, :],
                                    start=(c == 0), stop=(c == NCH - 1),
                                )
                            nc.scalar.mul(out=qT_sb[b][:, h, t0:t0 + TT], in_=ps[:], mul=1.0)
                        ps = ppsum.tile([P, TT], f32, tag="pp")
                        for c in range(NCH):
                            nc.tensor.matmul(
                                ps[:], lhsT=wk_sb[:, c, :], rhs=xts[c // XSUB][:, c % XSUB, :],
                                start=(c == 0), stop=(c == NCH - 1),
                            )
                        nc.scalar.mul(out=kT_sb[b][:, t0:t0 + TT], in_=ps[:], mul=1.0)
                        ps = ppsum.tile([P, TT], f32, tag="pp")
                        for c in range(NCH):
                            nc.tensor.matmul(
                                ps[:], lhsT=wv_sb[:, c, :], rhs=xts[c // XSUB][:, c % XSUB, :],
                                start=(c == 0), stop=(c == NCH - 1),
                            )
                        vt = vstage.tile([P, TT], f32r, tag="vt")
                        nc.vector.tensor_copy(vt[:], ps[:])
                        for j in range(TT // P):
                            tp = tpsum.tile([P, P], f32r, tag="tp")
                            nc.tensor.transpose(tp[:], vt[:, j * P:(j + 1) * P], ident[:])
                            kc = t0 // P + j
                            nc.vector.tensor_copy(v_sb[b][:, kc * HD: (kc + 1) * HD], tp[:])

            # ---------------- phase 2+3: attention ----------------
            with tc.tile_pool(name="mask", bufs=4) as mpool, \
                 tc.tile_pool(name="et", bufs=1) as epool, \
                 tc.tile_pool(name="ot", bufs=2) as otpool, \
                 tc.tile_pool(name="small", bufs=8) as small, \
                 tc.tile_pool(name="spsum", bufs=3, space="PSUM") as spsum, \
                 tc.tile_pool(name="opsum", bufs=2, space="PSUM") as opsum, \
                 tc.tile_pool(name="supsum", bufs=1, space="PSUM") as supsum, \
                 tc.tile_pool(name="tpsum2", bufs=1, space="PSUM") as tpsum2:
                for b in range(B):
                    for q0 in range(0, S, QT):
                        act = active_kchunks(q0)
                        msk = masked_kchunks(q0)
                        et = [epool.tile([P, NKC, QT], f32r, tag=f"et{h}", name=f"et{h}_{b}_{q0}")
                              for h in range(HPC)]
                        # scores + exp, per k-chunk (mask tile shared by both heads)
                        for kc in act:
                            if kc in msk:
                                mt = mpool.tile([P, QT], bf16, tag="mt")
                                nc.sync.dma_start(
                                    out=mt[:], in_=maskT[b, kc * P:(kc + 1) * P, q0:q0 + QT]
                                )
                            for h in range(HPC):
                                sp = spsum.tile([P, QT], f32, tag="sp")
                                nc.tensor.matmul(
                                    sp[:],
                                    lhsT=kT_sb[b][:, kc * P:(kc + 1) * P],
                                    rhs=qT_sb[b][:, h, q0:q0 + QT],
                                    start=True, stop=True,
                                )
                                if kc in msk:
                                    nc.vector.tensor_add(out=sp[:], in0=sp[:], in1=mt[:])
                                nc.scalar.activation(out=et[h][:, kc, :], in_=sp[:], func=Exp)
                        # attn @ V (out^T form)
                        po = {}
                        for h in range(HPC):
                            po[h] = opsum.tile([P, QT], f32, tag="po", name=f"po{h}_{b}_{q0}")
                            for kc in act:
                                nc.tensor.matmul(
                                    po[h][:],
                                    lhsT=v_sb[b][:, kc * HD:(kc + 1) * HD],
                                    rhs=et[h][:, kc, :],
                                    start=(kc == act[0]), stop=(kc == act[-1]),
                                )
                        srow2 = small.tile([32 * (HPC - 1) + 1, QT], f32, tag="srow2")
                        oT = {}
                        for h in range(HPC):
                            psums = supsum.tile([P, QT], f32, tag="ps_sums")
                            for kc in act:
                                nc.tensor.matmul(
                                    psums[:1, :],
                                    lhsT=ones_sb[:, :1],
                                    rhs=et[h][:, kc, :],
                                    start=(kc == act[0]), stop=(kc == act[-1]),
                                )
                            oT[h] = otpool.tile([P, QT], f32, tag=f"oT{h}", name=f"oT{h}_{b}_{q0}")
                            nc.scalar.mul(out=oT[h][:], in_=po[h][:], mul=1.0)
                            nc.scalar.mul(out=srow2[32 * h:32 * h + 1, :], in_=psums[:1, :], mul=1.0)
                        # flip [d, q] -> [q, d] and normalize per-q
                        for qj in range(QT // P):
                            tps = tpsum2.tile([P, P], f32, tag="tps")
                            nw = 32 * (HPC - 1) + 1
                            nc.tensor.transpose(
                                tps[:, :nw], srow2[:, qj * P:(qj + 1) * P],
                                ident32[:nw, :nw],
                            )
                            rc = small.tile([P, HPC], f32, tag="rc")
                            for h in range(HPC):
                                nc.vector.reciprocal(rc[:, h:h + 1], tps[:, 32 * h:32 * h + 1])
                            for h in range(HPC):
                                tpo = tpsum2.tile([P, P], f32, tag="tpo")
                                nc.tensor.transpose(
                                    tpo[:], oT[h][:, qj * P:(qj + 1) * P], ident32[:]
                                )
                                ob = small.tile([P, HD], f32, tag="ob")
                                nc.vector.tensor_scalar_mul(ob[:], tpo[:], rc[:, h:h + 1])
                                nc.sync.dma_start(
                                    out=out[b, q0 + qj * P: q0 + (qj + 1) * P, h * HD:(h + 1) * HD],
                                    in_=ob[:],
                                )

    nc.compile()
    return nc


def get_nc(variant="general"):
    if variant not in _CACHE:
        _CACHE[variant] = _build_nc(variant)
    return _CACHE[variant]


def detect_variant(attention_mask):
    m = np.asarray(attention_mask, dtype=np.float32)[:, 0]   # [B, S, S] (q, k)
    if not np.any(m):
        return "zeros"
    # causal: zero on/below the diagonal, <= -1e8 strictly above
    kk = np.arange(S)
    lower = kk[None, :] <= kk[:, None]                       # [S(q), S(k)]
    for b in range(m.shape[0]):
        if np.any(m[b][lower] != 0.0):
            return "general"
        if np.any(m[b][~lower] > -1e8):
            return "general"
    return "causal"


def make_in_maps(hidden_states, attention_mask, Wq, Wk, Wv):
    import ml_dtypes

    xT = np.ascontiguousarray(
        np.asarray(hidden_states, dtype=np.float32).transpose(0, 2, 1)
    )
    mT = np.ascontiguousarray(
        np.asarray(attention_mask, dtype=np.float32)[:, 0].transpose(0, 2, 1)
    ).astype(ml_dtypes.bfloat16)
    wq_s = (np.asarray(Wq, dtype=np.float32) / math.sqrt(HD)).astype(np.float32)
    wk = np.asarray(Wk, dtype=np.float32)
    wv = np.asarray(Wv, dtype=np.float32)
    ident = np.eye(P, dtype=np.float32)
    ones = np.ones((P, 1), dtype=np.float32)

    in_maps = []
    for c in range(NCORES):
        kv = c // 2
        in_maps.append({
            "xT": xT,
            "wq": np.ascontiguousarray(wq_s[:, c * FPC:(c + 1) * FPC]),
            "wk": np.ascontiguousarray(wk[:, kv * KVW:(kv + 1) * KVW]),
            "wv": np.ascontiguousarray(wv[:, kv * KVW:(kv + 1) * KVW]),
            "maskT": mT,
            "ident": ident,
            "ident32": ident,
            "ones": ones,
        })
    return in_maps


def kernel(hidden_states, attention_mask, Wq, Wk, Wv):
    from concourse.bass_utils import run_bass_kernel_spmd

    variant = detect_variant(attention_mask)
    nc = get_nc(variant)
    in_maps = make_in_maps(hidden_states, attention_mask, Wq, Wk, Wv)
    res = run_bass_kernel_spmd(nc, in_maps, core_ids=list(range(NCORES)))
    outs = [res.results[c]["out"] for c in range(NCORES)]
    return np.concatenate(outs, axis=2).astype(np.float32)



# revision 7
# speedup vs baseline: 1.6464x; 1.6464x over previous
"""Trainium2 Bass kernel for CheemsNonWoAttention (GQA attention, no output proj).

Sharding (v3): one (kv-head, batch) pair per core — 4 kv heads x 2 batches =
8 cores.  Each core projects K/V for its kv head and Q for the 4 q-heads of
that GQA group, over ONE batch only.  No work is duplicated anywhere (the
per-core PE load sits at the 8-way MAC roofline) and no collectives are
needed; the host concatenates per-core output slices.

Kernel design:
  - All matmuls bf16 (same 1 cycle/row PE rate as fp32r at moving>=256, but
    full rate at ANY even moving width; DMA and SBUF traffic halved).
  - Scores computed transposed, sT[k, q], per 512-q block; per k-chunk the
    moving range starts at max(q0, kc*128) (causal), so the upper-left
    rectangle of each diagonal block is never computed.  The remaining
    128x128 triangle is zeroed by a 0/1 multiply on the exp output.
  - exp on the scalar engine over 2-bank PSUM groups (FD=1024) to amortize
    the ~293ns per-ACTIVATE overhead.
  - Softmax denominators: partial sums over k accumulated on the vector
    engine in bf16 (2x mode) into two per-head panels (k-chunks 0-7 and
    8-15, capping accumulation depth at 8 for precision), DMA'd out raw;
    the HOST does the final 128-way partition reduction and the divide.
    No sums matmuls and no output transposes on the PE at all.
  - attn@V output staged PSUM->SBUF as bf16 (copies alternate between the
    scalar and vector engines) and DMA'd as oT[d, q]; host transposes and
    normalizes.
  - Causal pipelining: attention on q-block t0 only needs K/V for tokens
    <= t0+512, so each projection tile is immediately followed by its
    attention block and the engines overlap across the whole kernel.
"""

import sys

if "/opt/trn_rl_repo" not in sys.path:
    sys.path.insert(0, "/opt/trn_rl_repo")

import math
import numpy as np

B, S, HID = 2, 2048, 2048
NH, NKV, HD = 16, 4, 128
NCORES = 8
HPC = 4                     # q heads per core (one GQA group)
FPC = HPC * HD              # 512 output features per core
KVW = HD                    # kv head width per core
P = 128
NCH = HID // P              # hid chunks (contraction tiles)
TT = 512                    # token tile, phase 1
QT = 512                    # q block, phase 2
NKC = S // P                # k chunks
GK = 2                      # k-chunks per exp group (2 PSUM banks)
XSUB = 4                    # hid chunks per x sub-tile
NSUB = NCH // XSUB

_CACHE = {}


def _patch_ldw_opt():
    # walrus's LDWEIGHTS dedup/overlap pass is off by default in the driver
    # args; weight loads otherwise throttle back-to-back matmul issue.
    import concourse.bass_utils as bu

    if getattr(bu, "_ldw_opt_patched", False):
        return
    orig = bu.run_command

    def patched(argv, **kw):
        argv = ["--enable-ldw-opt=true" if a == "--enable-ldw-opt=false" else a
                for a in argv]
        return orig(argv, **kw)

    bu.run_command = patched
    bu._ldw_opt_patched = True


def _build_nc(variant):
    # NOTE: ldw-opt is left OFF — walrus rejects it for bf16 LDWEIGHTS
    # ("InstLdweights is not compatible with LDW optimization"); bf16 weight
    # loads get the compiler-automatic FWL fast path instead.
    import concourse.bacc as bacc
    from concourse import mybir
    from concourse.tile import TileContext

    f32 = mybir.dt.float32
    f32r = mybir.dt.float32r
    bf16 = mybir.dt.bfloat16
    Exp = mybir.ActivationFunctionType.Exp

    nc = bacc.Bacc("TRN2", target_bir_lowering=False, debug=False, num_devices=NCORES)
    xT = nc.dram_tensor("xT", [HID, S], bf16, kind="ExternalInput").ap()
    wq = nc.dram_tensor("wq", [HID, FPC], bf16, kind="ExternalInput").ap()
    wk = nc.dram_tensor("wk", [HID, KVW], bf16, kind="ExternalInput").ap()
    wv = nc.dram_tensor("wv", [HID, KVW], bf16, kind="ExternalInput").ap()
    identb_d = nc.dram_tensor("identb", [P, P], f32r, kind="ExternalInput").ap()
    tri_d = nc.dram_tensor("tri", [P, P], bf16, kind="ExternalInput").ap()
    if variant == "general":
        maskT = nc.dram_tensor("maskT", [S, S], bf16, kind="ExternalInput").ap()
    oT_d = nc.dram_tensor("oT", [FPC, S], bf16, kind="ExternalOutput").ap()
    accL_d = nc.dram_tensor("accL", [HPC, P, S], bf16, kind="ExternalOutput").ap()
    accH_d = nc.dram_tensor("accH", [HPC, P, S], bf16, kind="ExternalOutput").ap()

    def n_kchunks(q0):
        if variant == "causal":
            return q0 // P + QT // P
        return NKC

    def q_start(q0, kc):
        # first valid (unmasked) q column for this k-chunk
        if variant == "causal":
            return max(q0, kc * P)
        return q0

    with TileContext(nc) as tc:
        with tc.tile_pool(name="persist", bufs=1) as persist, \
             tc.tile_pool(name="xt", bufs=2) as xpool, \
             tc.tile_pool(name="vst", bufs=2) as vstage, \
             tc.tile_pool(name="ost", bufs=3) as ostage, \
             tc.tile_pool(name="et", bufs=3) as etpool, \
             tc.tile_pool(name="acc", bufs=2 * HPC) as accpool, \
             tc.tile_pool(name="mask", bufs=4) as mpool, \
             tc.tile_pool(name="ppsum", bufs=2, space="PSUM") as ppsum, \
             tc.tile_pool(name="spsum", bufs=2, space="PSUM") as spsum, \
             tc.tile_pool(name="popsum", bufs=2, space="PSUM") as popool:
            wq_sb = persist.tile([P, NCH, FPC], bf16, tag="wq")
            wk_sb = persist.tile([P, NCH, KVW], bf16, tag="wk")
            wv_sb = persist.tile([P, NCH, KVW], bf16, tag="wv")
            identb = persist.tile([P, P], f32r, tag="identb")
            tri_sb = persist.tile([P, P], bf16, tag="tri")
            qT_sb = persist.tile([P, HPC, S], bf16, tag="qT")
            kT_sb = persist.tile([P, S], bf16, tag="kT")
            v_sb = persist.tile([P, S], bf16, tag="v")

            nc.sync.dma_start(out=wq_sb[:], in_=wq.rearrange("(c p) f -> p c f", p=P))
            nc.sync.dma_start(out=wk_sb[:], in_=wk.rearrange("(c p) f -> p c f", p=P))
            nc.sync.dma_start(out=wv_sb[:], in_=wv.rearrange("(c p) f -> p c f", p=P))
            nc.gpsimd.dma_start(out=identb[:], in_=identb_d[:])
            nc.gpsimd.dma_start(out=tri_sb[:], in_=tri_d[:])

            def proj_tile(t0):
                xts = []
                for s in range(NSUB):
                    xs = xpool.tile([P, XSUB, TT], bf16, tag=f"xt{s}",
                                    name=f"xt{s}_{t0}")
                    eng = nc.sync if s % 2 == 0 else nc.gpsimd
                    eng.dma_start(
                        out=xs[:],
                        in_=xT[s * XSUB * P:(s + 1) * XSUB * P, t0:t0 + TT]
                        .rearrange("(c p) t -> p c t", p=P),
                    )
                    xts.append(xs)
                for h in range(HPC):
                    ps = ppsum.tile([P, TT], f32, tag="pp")
                    for c in range(NCH):
                        nc.tensor.matmul(
                            ps[:],
                            lhsT=wq_sb[:, c, h * HD:(h + 1) * HD],
                            rhs=xts[c // XSUB][:, c % XSUB, :],
                            start=(c == 0), stop=(c == NCH - 1),
                        )
                    nc.vector.tensor_copy(qT_sb[:, h, t0:t0 + TT], ps[:])
                ps = ppsum.tile([P, TT], f32, tag="pp")
                for c in range(NCH):
                    nc.tensor.matmul(
                        ps[:], lhsT=wk_sb[:, c, :], rhs=xts[c // XSUB][:, c % XSUB, :],
                        start=(c == 0), stop=(c == NCH - 1),
                    )
                nc.vector.tensor_copy(kT_sb[:, t0:t0 + TT], ps[:])
                ps = ppsum.tile([P, TT], f32, tag="pp")
                for c in range(NCH):
                    nc.tensor.matmul(
                        ps[:], lhsT=wv_sb[:, c, :], rhs=xts[c // XSUB][:, c % XSUB, :],
                        start=(c == 0), stop=(c == NCH - 1),
                    )
                vt = vstage.tile([P, TT], f32r, tag="vt", name=f"vt_{t0}")
                nc.scalar.copy(vt[:], ps[:])
                vtp = popool.tile([P, QT], f32r, tag="po", name=f"vtp_{t0}")
                for j in range(TT // P):
                    nc.tensor.transpose(
                        vtp[:, j * P:(j + 1) * P], vt[:, j * P:(j + 1) * P], identb[:]
                    )
                nc.vector.tensor_copy(v_sb[:, t0:t0 + TT], vtp[:])

            acc_tiles = {}

            def attn_block(q0):
                nkc = n_kchunks(q0)
                for h in range(HPC):
                    if h not in acc_tiles:
                        accL = accpool.tile([P, S], bf16, tag="acc", name=f"accL_{h}")
                        accH = accpool.tile([P, S], bf16, tag="acc", name=f"accH_{h}")
                        acc_tiles[h] = (accL, accH)
                    accL, accH = acc_tiles[h]
                    et = etpool.tile([P, NKC * QT], bf16, tag="et",
                                     name=f"et_{q0}_{h}")
                    # ---- scores + exp, in GK-chunk groups ----
                    for g0 in range(0, nkc, GK):
                        sp = spsum.tile([P, GK * QT], f32, tag="sp")
                        for kc in range(g0, g0 + GK):
                            qs = q_start(q0, kc)
                            off = (kc - g0) * QT + (qs - q0)
                            nc.tensor.matmul(
                                sp[:, off:(kc - g0) * QT + QT],
                                lhsT=kT_sb[:, kc * P:(kc + 1) * P],
                                rhs=qT_sb[:, h, qs:q0 + QT],
                                start=True, stop=True,
                            )
                        if variant == "general":
                            mt = mpool.tile([P, GK, QT], bf16, tag="mt")
                            nc.sync.dma_start(
                                out=mt[:],
                                in_=maskT[g0 * P:(g0 + GK) * P, q0:q0 + QT]
                                .rearrange("(g p) q -> p g q", p=P),
                            )
                            nc.vector.tensor_add(out=sp[:], in0=sp[:], in1=mt[:])
                        nc.scalar.activation(
                            out=et[:, g0 * QT:(g0 + GK) * QT], in_=sp[:], func=Exp
                        )
                        if variant == "causal":
                            for kc in range(g0, g0 + GK):
                                if kc * P >= q0:
                                    off = kc * QT + (kc * P - q0)
                                    nc.vector.tensor_mul(
                                        out=et[:, off:off + P],
                                        in0=et[:, off:off + P],
                                        in1=tri_sb[:],
                                    )
                    # ---- attn @ V ----
                    po = popool.tile([P, QT], f32, tag="po", name=f"po_{q0}_{h}")
                    for kc in range(nkc):
                        qs = q_start(q0, kc)
                        off = qs - q0
                        nc.tensor.matmul(
                            po[:, off:QT],
                            lhsT=v_sb[:, kc * P:(kc + 1) * P],
                            rhs=et[:, kc * QT + off:kc * QT + QT],
                            start=(kc == 0), stop=(kc == nkc - 1),
                        )
                    ot = ostage.tile([P, QT], bf16, tag="ot", name=f"ot_{q0}_{h}")
                    if h % 2 == 0:
                        nc.scalar.copy(ot[:], po[:])
                    else:
                        nc.vector.tensor_copy(ot[:], po[:])
                    nc.gpsimd.dma_start(
                        out=oT_d[h * P:(h + 1) * P, q0:q0 + QT], in_=ot[:]
                    )
                    # ---- softmax partial sums on DVE (bf16, depth <= 8) ----
                    for kc in range(nkc):
                        qs = q_start(q0, kc)
                        off = qs - q0
                        acc = accL if kc < 8 else accH
                        src = et[:, kc * QT + off:kc * QT + QT]
                        dst = acc[:, qs:q0 + QT]
                        if kc == 0 or kc == 8:
                            nc.vector.tensor_copy(dst, src)
                        else:
                            nc.vector.tensor_add(out=dst, in0=dst, in1=src)
                    if q0 == S - QT:
                        nc.gpsimd.dma_start(out=accL_d[h], in_=accL[:])
                        nc.gpsimd.dma_start(out=accH_d[h], in_=accH[:])

            # causal pipeline: attention on block t0 needs only tokens <= t0+TT
            for t0 in range(0, S, TT):
                proj_tile(t0)
                attn_block(t0)

    nc.compile()
    return nc


def get_nc(variant="causal"):
    if variant not in _CACHE:
        _CACHE[variant] = _build_nc(variant)
    return _CACHE[variant]


def detect_variant(attention_mask):
    m = np.asarray(attention_mask, dtype=np.float32)[:, 0]   # [B, S, S] (q, k)
    if not np.any(m):
        return "zeros"
    kk = np.arange(S)
    lower = kk[None, :] <= kk[:, None]                       # [S(q), S(k)]
    for b in range(m.shape[0]):
        if np.any(m[b][lower] != 0.0):
            return "general"
        if np.any(m[b][~lower] > -1e8):
            return "general"
    return "causal"


def make_in_maps(hidden_states, attention_mask, Wq, Wk, Wv, variant=None):
    import ml_dtypes

    if variant is None:
        variant = detect_variant(attention_mask)
    bf = ml_dtypes.bfloat16
    x = np.asarray(hidden_states, dtype=np.float32)
    xTb = [np.ascontiguousarray(x[b].T).astype(bf) for b in range(B)]
    wq_s = (np.asarray(Wq, dtype=np.float32) / math.sqrt(HD)).astype(bf)
    wk = np.asarray(Wk, dtype=np.float32).astype(bf)
    wv = np.asarray(Wv, dtype=np.float32).astype(bf)
    identb = np.eye(P, dtype=np.float32)
    # tri[p, j] = 1 if p <= j else 0  (keep k <= q within the diagonal chunk)
    tri = np.triu(np.ones((P, P), dtype=np.float32)).astype(bf)
    if variant == "general":
        mT = [np.ascontiguousarray(
            np.asarray(attention_mask, dtype=np.float32)[b, 0].T).astype(bf)
            for b in range(B)]

    in_maps = []
    for c in range(NCORES):
        b, kv = c % 2, c // 2
        m = {
            "xT": xTb[b],
            "wq": np.ascontiguousarray(wq_s[:, kv * FPC:(kv + 1) * FPC]),
            "wk": np.ascontiguousarray(wk[:, kv * KVW:(kv + 1) * KVW]),
            "wv": np.ascontiguousarray(wv[:, kv * KVW:(kv + 1) * KVW]),
            "identb": identb,
            "tri": tri,
        }
        if variant == "general":
            m["maskT"] = mT[b]
        in_maps.append(m)
    return in_maps


def postprocess(res, variant):
    """Assemble full [B, S, HID] f32 output from per-core oT/accL/accH."""
    out = np.empty((B, S, HID), dtype=np.float32)
    for c in range(NCORES):
        b, kv = c % 2, c // 2
        oT = res.results[c]["oT"].astype(np.float64)          # [FPC, S]
        aL = res.results[c]["accL"].astype(np.float64)        # [HPC, P, S]
        aH = res.results[c]["accH"].astype(np.float64)
        if variant == "causal":
            # k-chunks 8..15 only reach q >= 1024; cols below hold garbage
            aH = aH.copy()
            aH[..., :8 * P] = 0.0
        sums = aL.sum(axis=1) + aH.sum(axis=1)                # [HPC, S]
        o = (oT.reshape(HPC, HD, S) / sums[:, None, :])       # [HPC, HD, S]
        out[b, :, kv * FPC:(kv + 1) * FPC] = (
            o.transpose(2, 0, 1).reshape(S, FPC).astype(np.float32)
        )
    return out


def run_on_cores(inputs, trace=False, tmpdir=None):
    from concourse.bass_utils import run_bass_kernel_spmd

    variant = detect_variant(inputs["attention_mask"])
    nc = get_nc(variant)
    in_maps = make_in_maps(**inputs, variant=variant)
    kw = {}
    if trace:
        kw = {"trace": True, "tmpdir": tmpdir}
    res = run_bass_kernel_spmd(nc, in_maps, core_ids=list(range(NCORES)), **kw)
    return postprocess(res, variant), res


def kernel(hidden_states, attention_mask, Wq, Wk, Wv):
    out, _ = run_on_cores({
        "hidden_states": hidden_states,
        "attention_mask": attention_mask,
        "Wq": Wq, "Wk": Wk, "Wv": Wv,
    })
    return out
